# Initial kernel scaffold
#
"""MHSA + BatchNorm + residual for Trainium2, SPMD across 8 NeuronCores.

Problem (hardcoded): x [B=2, C=1024, T=2048] fp32
  q/k/v = W @ x[b] + b  (1x1 conv, per batch)
  16 heads x 64 dims, softmax attention over T
  y = Wo @ out + bo ; BatchNorm1d over (B, T); return x + gamma*norm(y)+beta

Sharding: 8 cores = 2 batches x 4 t-slices of 512 query positions.
Each core computes K / V^T for its FULL batch (4x redundant; cheaper than
gathering 16MB over the on-chip fabric at ~60GB/s) and Q only for its
t-slice. Attention + Wo + BN-apply are fully local; only the BatchNorm
mean/var reduction needs cross-core traffic (a [128,16] AllReduce).

dtypes: float32r (TF32-like, fp32 bytes) for the projection/Wo matmuls;
bf16 for K/Q/V'/E inside attention (SBUF budget); fp32 PSUM everywhere.
"""

import numpy as np

import concourse.bass as bass
import concourse.mybir as mybir
import concourse.tile as tile
from concourse import bacc
from concourse.bass_utils import run_bass_kernel_spmd

# problem dims
B, C, T, H, DH = 2, 1024, 2048, 16, 64
P = 128
KO = C // P            # 8 channel tiles
TS = 512               # t-slice per core
NT = T // P            # 16 s-tiles
SCALE = DH ** -0.5     # 0.125
EPS = 1e-5
NCORES = 8
NBT = B * T            # BatchNorm count

F32 = mybir.dt.float32
F32R = mybir.dt.float32r
BF16 = mybir.dt.bfloat16

TRACE = False          # test.py flips this for profiling
LAST_RESULT = None     # BassKernelResults of the last run

_cached_nc = None


def _build():
    nc = bacc.Bacc("TRN2", target_bir_lowering=False, debug=False,
                   num_devices=NCORES)

    xkv_d = nc.dram_tensor("xkv", [C, T], F32R, kind="ExternalInput").ap()
    xq_d = nc.dram_tensor("xq", [C, TS], F32R, kind="ExternalInput").ap()
    wqT_d = nc.dram_tensor("wqT", [C, C], F32R, kind="ExternalInput").ap()
    wkT_d = nc.dram_tensor("wkT", [C, C], F32R, kind="ExternalInput").ap()
    wvT_d = nc.dram_tensor("wvT", [C, C], F32R, kind="ExternalInput").ap()
    woT_d = nc.dram_tensor("woT", [C, C], F32R, kind="ExternalInput").ap()
    bq_d = nc.dram_tensor("bq", [C], F32, kind="ExternalInput").ap()
    bk_d = nc.dram_tensor("bk", [C], F32, kind="ExternalInput").ap()
    bv_d = nc.dram_tensor("bv", [C], F32, kind="ExternalInput").ap()
    bo_d = nc.dram_tensor("bo", [C], F32, kind="ExternalInput").ap()
    gamma_d = nc.dram_tensor("gamma", [C], F32, kind="ExternalInput").ap()
    sel_d = nc.dram_tensor("sel", [H, C], F32R, kind="ExternalInput").ap()
    beta_d = nc.dram_tensor("beta", [C], F32, kind="ExternalInput").ap()
    out_d = nc.dram_tensor("out", [C, TS], F32, kind="ExternalOutput").ap()

    # [C] -> [P, KO] so channel c sits at (partition c%128, free c//128)
    def chan_vec(ap):
        return ap.rearrange("(o p) -> p o", p=P)

    def chan_mat(ap):
        return ap.rearrange("(o p) n -> p o n", p=P)

    with tile.TileContext(nc) as tc:
        with (
            tc.tile_pool(name="consts", bufs=1) as consts,
            tc.tile_pool(name="persist", bufs=1) as persist,
            tc.tile_pool(name="dram", bufs=1, space="DRAM") as drampool,
        ):
            # ---- constants ----
            bq_sb = consts.tile([P, KO], F32, name="bq_sb")
            nc.sync.dma_start(bq_sb[:], chan_vec(bq_d))
            bk_sb = consts.tile([P, KO], F32, name="bk_sb")
            nc.sync.dma_start(bk_sb[:], chan_vec(bk_d))
            bo_sb = consts.tile([P, KO], F32, name="bo_sb")
            nc.sync.dma_start(bo_sb[:], chan_vec(bo_d))
            gamma_sb = consts.tile([P, KO], F32, name="gamma_sb")
            nc.sync.dma_start(gamma_sb[:], chan_vec(gamma_d))
            beta_sb = consts.tile([P, KO], F32, name="beta_sb")
            nc.sync.dma_start(beta_sb[:], chan_vec(beta_d))
            bvc_sb = consts.tile([P, KO], F32, name="bvc_sb")
            nc.sync.dma_start(bvc_sb[:], chan_vec(bv_d))
            eps_sb = consts.tile([P, 1], F32, name="eps_sb")
            nc.vector.memset(eps_sb[:], EPS)
            # sel[h, (o, p)] = 1 iff channel (o,p) belongs to head h
            sel_sb = consts.tile([H, KO, P], F32R, name="sel_sb")
            nc.sync.dma_start(
                sel_sb[:], sel_d.rearrange("h (o p) -> h o p", p=P))

            # ---- persistent activations (live through phase D) ----
            xq_sb = persist.tile([P, KO, TS], F32R, name="xq_sb")
            nc.sync.dma_start(xq_sb[:], chan_mat(xq_d))
            k_sb = persist.tile([P, KO, T], BF16, name="k_sb")
            q_sb = persist.tile([P, KO, TS], BF16, name="q_sb")
            # V' [t-part, t-tile, head, DH+1]; col DH is the ones column
            vp_sb = persist.tile([P, NT, H, DH + 1], BF16, name="vp_sb")
            s1_sb = persist.tile([P, KO], F32, name="s1_sb")
            s2_sb = persist.tile([P, KO], F32, name="s2_sb")

            # ---- phases A+B: projections (wq/wk/wv live here only) ----
            with (
                tc.tile_pool(name="wpool", bufs=16) as wpool,
                tc.tile_pool(name="ppsum", bufs=4, space="PSUM") as ppsum,
            ):
                def load_w_chunks(w_d):
                    tiles = []
                    for ki in range(KO):
                        t = wpool.tile([P, C], F32R, name="w_sb", tag="w")
                        nc.sync.dma_start(t[:], chan_mat(w_d)[:, ki, :])
                        tiles.append(t)
                    return tiles
                wq_t = load_w_chunks(wqT_d)
                wk_t = load_w_chunks(wkT_d)

                # Q projection (needs xq only)
                for o in range(KO):
                    ps = ppsum.tile([P, TS], F32, name="qk_ps", tag="pp")
                    for ki in range(KO):
                        nc.tensor.matmul(
                            ps[:],
                            wq_t[ki][:, o * P:(o + 1) * P],
                            xq_sb[:, ki, :],
                            start=(ki == 0), stop=(ki == KO - 1),
                        )
                    nc.scalar.activation(
                        q_sb[:, o, :], ps[:],
                        mybir.ActivationFunctionType.Identity,
                        bias=bq_sb[:, o:o + 1])

                wv_t = None

                # K and V^T over the full batch, in t-quarters
                with tc.tile_pool(name="xkvp", bufs=2) as xkvp:
                    for quarter in range(4):
                        t0 = quarter * TS
                        xh = xkvp.tile([P, KO, TS], F32R,
                                       name="xkv_sb", tag="xkv")
                        nc.sync.dma_start(
                            xh[:], chan_mat(xkv_d)[:, :, t0:t0 + TS])
                        if wv_t is None:
                            # reuses wq's slots (waits for Q-proj reads)
                            wv_t = load_w_chunks(wvT_d)
                        # K chan-major
                        for o in range(KO):
                            ps = ppsum.tile([P, TS], F32,
                                            name="qk_ps", tag="pp")
                            for ki in range(KO):
                                nc.tensor.matmul(
                                    ps[:],
                                    wk_t[ki][:, o * P:(o + 1) * P],
                                    xh[:, ki, :],
                                    start=(ki == 0), stop=(ki == KO - 1),
                                )
                            nc.scalar.activation(
                                k_sb[:, o, t0:t0 + TS], ps[:],
                                mybir.ActivationFunctionType.Identity,
                                bias=bk_sb[:, o:o + 1])
                        # V^T t-major: psum [t-tile, 512 chans]
                        for tt in range(TS // P):
                            for nch in range(2):
                                ps = ppsum.tile([P, TS], F32,
                                                name="qk_ps", tag="pp")
                                for ki in range(KO):
                                    nc.tensor.matmul(
                                        ps[:],
                                        xh[:, ki, tt * P:(tt + 1) * P],
                                        wv_t[ki][:,
                                                 nch * TS:(nch + 1) * TS],
                                        start=(ki == 0), stop=(ki == KO - 1),
                                    )
                                nc.vector.tensor_copy(
                                    vp_sb[:, quarter * 4 + tt,
                                          nch * 8:(nch + 1) * 8, 0:DH],
                                    ps[:].rearrange(
                                        "p (h d) -> p h d", d=DH))
                        nc.vector.memset(
                            vp_sb[:, quarter * 4:(quarter + 1) * 4,
                                  :, DH:DH + 1],
                            1.0)

            # ---- phases C+D: attention, Wo, stats ----
            with (
                tc.tile_pool(name="wopool", bufs=1) as wopool,
                tc.tile_pool(name="actp", bufs=1) as actp,
                tc.tile_pool(name="hpool", bufs=2) as hpool,
            ):
                wo_sb = wopool.tile([P, KO, C], F32R, name="wo_sb")
                nc.sync.dma_start(wo_sb[:], chan_mat(woT_d))
                out_sb = actp.tile([P, KO, TS], F32R, name="out_sb")
                den_all = actp.tile([H, TS], F32R, name="den_all")

                CH = 3  # s-tiles per exp chunk
                chunks = [(s, min(CH, NT - s)) for s in range(0, NT, CH)]
                with (
                    tc.tile_pool(name="epool", bufs=2) as epool,
                    tc.tile_pool(name="spsum", bufs=2, space="PSUM") as spsum,
                    tc.tile_pool(name="apsum", bufs=2, space="PSUM") as apsum,
                ):
                    def emit_sims(h):
                        pb = DH * (h & 1)           # partition base 0/64
                        o = h // 2
                        e_sb = epool.tile([P, NT, TS], BF16,
                                          name="e_sb", tag="e")
                        for s0, clen in chunks:
                            ps = spsum.tile([P, CH, TS], F32,
                                            name="sim_ps", tag="sim")
                            for j in range(clen):
                                nc.tensor.matmul(
                                    ps[:, j, :],
                                    k_sb[pb:pb + DH, o,
                                         (s0 + j) * P:(s0 + j + 1) * P],
                                    q_sb[pb:pb + DH, o, :],
                                    start=True, stop=True,
                                )
                            nc.scalar.activation(
                                e_sb[:, s0:s0 + clen, :],
                                ps[:, 0:clen, :],
                                mybir.ActivationFunctionType.Exp,
                                scale=SCALE)
                        return e_sb

                    def emit_av(h, e_sb):
                        pb = DH * (h & 1)
                        o = h // 2
                        av = apsum.tile([DH + 1, TS], F32,
                                        name="av_ps", tag="av")
                        for st in range(NT):
                            nc.tensor.matmul(
                                av[:],
                                vp_sb[:, st, h, :],
                                e_sb[:, st, :],
                                start=(st == 0), stop=(st == NT - 1),
                            )
                        # stash raw numerator + 1/denominator
                        dstage = hpool.tile([1, TS], F32R,
                                            name="dstage", tag="dstage")
                        with nc.allow_low_precision(
                                reason="f32r out is fp32 bytes"):
                            nc.vector.tensor_copy(
                                out_sb[pb:pb + DH, o, :], av[0:DH, :])
                            nc.vector.reciprocal(
                                dstage[:], av[DH:DH + 1, :])
                        nc.sync.dma_start(den_all[h:h + 1, :], dstage[:])

                    # software-pipelined: AV(h-1) interleaves with sims(h)
                    prev = None
                    for h in range(H):
                        e_cur = emit_sims(h)
                        if prev is not None:
                            emit_av(prev[0], prev[1])
                        prev = (h, e_cur)
                    emit_av(prev[0], prev[1])

                    # batched softmax division: per-o broadcast matmul
                    # of the per-head reciprocals + multiply + bias
                    for o in range(KO):
                        bc = apsum.tile([P, TS], F32, name="bc_ps", tag="av")
                        nc.tensor.matmul(
                            bc[:], sel_sb[:, o, :], den_all[:],
                            start=True, stop=True)
                        nc.vector.tensor_tensor(
                            out_sb[:, o, :], out_sb[:, o, :], bc[:],
                            mybir.AluOpType.mult)
                        nc.vector.tensor_scalar_add(
                            out_sb[:, o, :], out_sb[:, o, :],
                            bvc_sb[:, o:o + 1])

                # Wo projection + BN partial stats
                y_sb = actp.tile([P, KO, TS], F32, name="y_sb")
                with (
                    tc.tile_pool(name="ypsum", bufs=4, space="PSUM") as ypsum,
                    tc.tile_pool(name="scratch", bufs=2) as scratch,
                ):
                    for m in range(KO):
                        ps = ypsum.tile([P, TS], F32, name="y_ps", tag="yp")
                        for ki in range(KO):
                            nc.tensor.matmul(
                                ps[:],
                                wo_sb[:, ki, m * P:(m + 1) * P],
                                out_sb[:, ki, :],
                                start=(ki == 0), stop=(ki == KO - 1),
                            )
                        nc.scalar.activation(
                            y_sb[:, m, :], ps[:],
                            mybir.ActivationFunctionType.Identity,
                            bias=bo_sb[:, m:m + 1])
                        sq = scratch.tile([P, TS], F32, name="sq_sb", tag="sq")
                        nc.scalar.activation(
                            sq[:], y_sb[:, m, :],
                            mybir.ActivationFunctionType.Square,
                            accum_out=s2_sb[:, m:m + 1])
                        nc.vector.reduce_sum(
                            s1_sb[:, m:m + 1], y_sb[:, m, :],
                            axis=mybir.AxisListType.X)

                # ---- phase E: stats AllReduce + BN apply + residual ----
                stats_sb = hpool.tile([P, 2 * KO], F32, name="stats_sb",
                                      tag="stats")
                nc.vector.tensor_copy(stats_sb[:, 0:KO], s1_sb[:])
                nc.vector.tensor_copy(stats_sb[:, KO:2 * KO], s2_sb[:])
                st_in = drampool.tile([P, 2 * KO], F32, name="st_in")
                st_out = drampool.tile([NCORES, P, 2 * KO], F32,
                                       name="st_out")
                nc.sync.dma_start(st_in[:], stats_sb[:])
                nc.gpsimd.collective_compute(
                    "AllGather",
                    mybir.AluOpType.bypass,
                    replica_groups=[list(range(NCORES))],
                    ins=[st_in[:].opt()],
                    outs=[st_out[:].opt()],
                )
                gath_sb = hpool.tile([P, NCORES, 2 * KO], F32,
                                     name="gath_sb", tag="gath")
                nc.sync.dma_start(
                    gath_sb[:], st_out[:].rearrange("s p k -> p s k"))
                gstats_sb = hpool.tile([P, 2 * KO], F32, name="gstats_sb",
                                       tag="gstats")
                nc.vector.reduce_sum(
                    gstats_sb[:],
                    gath_sb[:].rearrange("p s k -> p k s"),
                    axis=mybir.AxisListType.X)

                mean_sb = hpool.tile([P, KO], F32, name="mean_sb", tag="mean")
                nc.vector.tensor_scalar_mul(
                    mean_sb[:], gstats_sb[:, 0:KO], 1.0 / NBT)
                var_sb = hpool.tile([P, KO], F32, name="var_sb", tag="var")
                nc.vector.tensor_scalar_mul(
                    var_sb[:], gstats_sb[:, KO:2 * KO], 1.0 / NBT)
                msq_sb = hpool.tile([P, KO], F32, name="msq_sb", tag="msq")
                nc.vector.tensor_tensor(
                    msq_sb[:], mean_sb[:], mean_sb[:], mybir.AluOpType.mult)
                nc.vector.tensor_tensor(
                    var_sb[:], var_sb[:], msq_sb[:], mybir.AluOpType.subtract)
                # rstd = 1/sqrt(var + eps)
                rstd_sb = hpool.tile([P, KO], F32, name="rstd_sb", tag="rstd")
                nc.scalar.activation(
                    rstd_sb[:], var_sb[:],
                    mybir.ActivationFunctionType.Sqrt, bias=eps_sb[:])
                nc.vector.reciprocal(rstd_sb[:], rstd_sb[:])
                # scale = gamma * rstd ; shift = beta - mean * scale
                scl_sb = hpool.tile([P, KO], F32, name="scl_sb", tag="scl")
                nc.vector.tensor_tensor(
                    scl_sb[:], gamma_sb[:], rstd_sb[:], mybir.AluOpType.mult)
                sh_sb = hpool.tile([P, KO], F32, name="sh_sb", tag="sh")
                nc.vector.tensor_tensor(
                    sh_sb[:], mean_sb[:], scl_sb[:], mybir.AluOpType.mult)
                nc.vector.tensor_tensor(
                    sh_sb[:], beta_sb[:], sh_sb[:], mybir.AluOpType.subtract)

                for m in range(KO):
                    nc.vector.tensor_scalar(
                        y_sb[:, m, :], y_sb[:, m, :],
                        scl_sb[:, m:m + 1], sh_sb[:, m:m + 1],
                        mybir.AluOpType.mult, mybir.AluOpType.add)
                    nc.vector.tensor_tensor(
                        y_sb[:, m, :], y_sb[:, m, :],
                        xq_sb[:, m, :].bitcast(F32), mybir.AluOpType.add)
                    nc.sync.dma_start(
                        chan_mat(out_d)[:, m, :], y_sb[:, m, :])

    nc.compile()
    return nc


def kernel(**inputs) -> np.ndarray:
    global _cached_nc, LAST_RESULT
    x = np.ascontiguousarray(inputs["x"], dtype=np.float32)
    wT = {k: np.ascontiguousarray(np.asarray(inputs[k]).T, dtype=np.float32)
          for k in ("Wq", "Wk", "Wv", "Wo")}
    vecs = {k: np.ascontiguousarray(inputs[k], dtype=np.float32)
            for k in ("bq", "bk", "bv", "bo", "gamma", "beta")}

    if _cached_nc is None:
        _cached_nc = _build()
    nc = _cached_nc

    sel = np.zeros((H, C), dtype=np.float32)
    for h in range(H):
        sel[h, h * DH:(h + 1) * DH] = 1.0

    in_maps = []
    for c in range(NCORES):
        b, t0 = c // 4, TS * (c % 4)
        in_maps.append({
            "xkv": x[b],
            "xq": np.ascontiguousarray(x[b][:, t0:t0 + TS]),
            "wqT": wT["Wq"], "wkT": wT["Wk"],
            "wvT": wT["Wv"], "woT": wT["Wo"],
            "sel": sel,
            "bq": vecs["bq"], "bk": vecs["bk"], "bv": vecs["bv"],
            "bo": vecs["bo"], "gamma": vecs["gamma"], "beta": vecs["beta"],
        })

    res = run_bass_kernel_spmd(
        nc, in_maps, core_ids=list(range(NCORES)), trace=TRACE)
    LAST_RESULT = res

    out = np.empty((B, C, T), dtype=np.float32)
    for c in range(NCORES):
        b, t0 = c // 4, TS * (c % 4)
        out[b][:, t0:t0 + TS] = res.results[c]["out"]
    return out



# revision 32
# speedup vs baseline: 1.0614x; 1.0614x over previous
"""MHSA + BatchNorm + residual for Trainium2, SPMD across 8 NeuronCores.

Problem (hardcoded): x [B=2, C=1024, T=2048] fp32
  q/k/v = W @ x[b] + b  (1x1 conv, per batch)
  16 heads x 64 dims, softmax attention over T
  y = Wo @ out + bo ; BatchNorm1d over (B, T); return x + gamma*norm(y)+beta

Sharding: 8 cores = 2 batches x 4 t-slices of 512 query positions.
v2 design vs baseline:
  - All activations/weights bf16 on the PE except the AV contraction,
    which runs fp8e4m3 in DoubleRow perf mode (2 k-tiles per pass,
    0.5 cycles/row): AV+den 131k -> 65k PE cycles.
  - E = exp(sim*scale - 2) emitted as fp8 directly by the Scalar engine
    (softmax is shift-invariant; -2 keeps E in fp8e4's finite range).
  - Softmax denominator via a [128,2,1]-ones DoubleRow matmul row; one
    batched reciprocal per head-pair instead of 16 serial [1,512]
    reciprocals (those cost 64us DVE in the baseline).
  - BatchNorm cross-core stats exchanged with remote_dma_broadcast
    (XOR slot per sender) instead of a collective AllGather: the
    collective cost 52us of pure tail latency for 8KB.
  - BN apply + residual split across DVE and GpSimd, per-tile output DMA.

dtypes: bf16 matmuls (1 cyc/row), fp8 DR for AV; fp32 PSUM everywhere.
"""

import numpy as np

import concourse.bass as bass
import concourse.mybir as mybir
import concourse.tile as tile
from concourse import bacc
from concourse.bass_utils import run_bass_kernel_spmd

# problem dims
B, C, T, H, DH = 2, 1024, 2048, 16, 64
P = 128
KO = C // P            # 8 channel tiles
TS = 512               # t-slice per core
NT = T // P            # 16 s-tiles
SCALE = DH ** -0.5     # 0.125
ESHIFT = -2.0          # exp shift; softmax-invariant, keeps E in fp8 range
EPS = 1e-5
NCORES = 8
NBT = B * T            # BatchNorm count

F32 = mybir.dt.float32
F32R = mybir.dt.float32r
BF16 = mybir.dt.bfloat16
FP8 = mybir.dt.float8e4
DR = mybir.MatmulPerfMode.DoubleRow

USE_REMOTE_STATS = False  # peer-DMA stats: walrus codegen rejects the
                          # remote-descs instructions in this toolchain

TRACE = False          # test.py flips this for profiling
LAST_RESULT = None     # BassKernelResults of the last run

_cached_nc = None


def _build():
    nc = bacc.Bacc("TRN2", target_bir_lowering=False, debug=False,
                   num_devices=NCORES)

    xkv_d = nc.dram_tensor("xkv", [C, T], BF16, kind="ExternalInput").ap()
    xq_d = nc.dram_tensor("xq", [C, TS], BF16, kind="ExternalInput").ap()
    xqf_d = nc.dram_tensor("xqf", [C, TS], F32, kind="ExternalInput").ap()
    wq_d = nc.dram_tensor("wq", [KO, P, C], BF16, kind="ExternalInput").ap()
    wk_d = nc.dram_tensor("wk", [KO, P, C], BF16, kind="ExternalInput").ap()
    wv_d = nc.dram_tensor("wv", [KO, P, C], BF16, kind="ExternalInput").ap()
    wo_d = nc.dram_tensor("wo", [KO, P, C], BF16, kind="ExternalInput").ap()
    bq_d = nc.dram_tensor("bq", [C], F32, kind="ExternalInput").ap()
    bk_d = nc.dram_tensor("bk", [C], F32, kind="ExternalInput").ap()
    bv_d = nc.dram_tensor("bv", [C], F32, kind="ExternalInput").ap()
    bo_d = nc.dram_tensor("bo", [C], F32, kind="ExternalInput").ap()
    gamma_d = nc.dram_tensor("gamma", [C], F32, kind="ExternalInput").ap()
    beta_d = nc.dram_tensor("beta", [C], F32, kind="ExternalInput").ap()
    out_d = nc.dram_tensor("out", [C, TS], F32, kind="ExternalOutput").ap()

    # [C] -> [P, KO] so channel c sits at (partition c%128, free c//128)
    def chan_vec(ap):
        return ap.rearrange("(o p) -> p o", p=P)

    with tile.TileContext(nc) as tc:
        if USE_REMOTE_STATS:
            rsem = nc.alloc_semaphore("stats_rsem")
            lsem = nc.alloc_semaphore("stats_lsem")

        with (
            tc.tile_pool(name="consts", bufs=1) as consts,
            tc.tile_pool(name="persist", bufs=1) as persist,
            tc.tile_pool(name="dram", bufs=1, space="DRAM") as drampool,
        ):
            # ---- persistent activations ----
            xq_sb = persist.tile([P, KO, TS], BF16, name="xq_sb")
            nc.sync.dma_start(xq_sb[:], xq_d.rearrange("(o p) n -> p o n", p=P))
            q_sb = persist.tile([P, KO, TS], BF16, name="q_sb")
            # K: [t-part, quarter, o, 512]
            kq_sb = persist.tile([P, 4, KO, TS], BF16, name="kq_sb")
            # V' fp8: [t-part, quarter, tt, head, DH]
            vp_sb = persist.tile([P, 4, 4, H, DH], FP8, name="vp_sb")
            num_sb = persist.tile([P, KO, TS], BF16, name="num_sb")
            y_sb = persist.tile([P, KO, TS], BF16, name="y_sb")
            # softmax denominators, packed on partition 0 (DVE partition-base
            # rules forbid per-head-partition writes)
            den_cat = persist.tile([1, H, TS], BF16, name="den_cat")
            rec_sb = persist.tile([P, TS], BF16, name="rec_sb")
            s1_sb = persist.tile([P, KO], F32, name="s1_sb")
            s2_sb = persist.tile([P, KO], F32, name="s2_sb")
            stats_sb = persist.tile([P, 2 * KO], F32, name="stats_sb")
            gst_sb = persist.tile([P, NCORES, 2 * KO], F32, name="gst_sb")
            sq_sb = persist.tile([P, TS], BF16, name="sq_sb")

            # warm up the collectives stack early (overlapped with phase
            # A/B): the first CC op of a NEFF pays ~15-30us of one-time
            # setup that would otherwise land on the stats-AllGather tail
            if not USE_REMOTE_STATS:
                warm_in = drampool.tile([1, 8], F32, name="warm_in")
                warm_out = drampool.tile([NCORES, 1, 8], F32,
                                         name="warm_out")
                nc.gpsimd.collective_compute(
                    "AllGather",
                    mybir.AluOpType.bypass,
                    replica_groups=[list(range(NCORES))],
                    ins=[warm_in[:].opt()],
                    outs=[warm_out[:].opt()],
                )

            # ---- constants ----
            bq_sb = consts.tile([P, KO], F32, name="bq_sb")
            nc.sync.dma_start(bq_sb[:], chan_vec(bq_d))
            bk_sb = consts.tile([P, KO], F32, name="bk_sb")
            nc.sync.dma_start(bk_sb[:], chan_vec(bk_d))
            bo_sb = consts.tile([P, KO], F32, name="bo_sb")
            nc.sync.dma_start(bo_sb[:], chan_vec(bo_d))
            gamma_sb = consts.tile([P, KO], F32, name="gamma_sb")
            nc.sync.dma_start(gamma_sb[:], chan_vec(gamma_d))
            beta_sb = consts.tile([P, KO], F32, name="beta_sb")
            nc.sync.dma_start(beta_sb[:], chan_vec(beta_d))
            bvc_sb = consts.tile([P, KO], F32, name="bvc_sb")
            nc.sync.dma_start(bvc_sb[:], chan_vec(bv_d))
            eps_sb = consts.tile([P, 1], F32, name="eps_sb")
            nc.vector.memset(eps_sb[:], EPS)
            eshift_sb = consts.tile([P, 1], F32, name="eshift_sb")
            nc.vector.memset(eshift_sb[:], ESHIFT)
            # DoubleRow ldweights needs the pair-dim step %16==0: pad to 16
            ones8_t = consts.tile([P, 2, 16], FP8, name="ones8")
            nc.vector.memset(ones8_t[:], 1.0)
            ones8 = ones8_t[:, :, 0:1]
            # [1, 128] ones row: broadcasts a partition-0 vector to 128
            # partitions via matmul (lhsT.T @ rhs with K=1)
            onesrow = consts.tile([1, P], BF16, name="onesrow")
            nc.vector.memset(onesrow[:], 1.0)
            # zero the remote-stats gather buffer before any peer can land
            nc.vector.memset(gst_sb[:], 0.0)

            xqf_sb = persist.tile([P, KO, TS], F32, name="xqf_sb")

            with (
                tc.tile_pool(name="wpool", bufs=1) as wpool,
                tc.tile_pool(name="xkvp", bufs=2) as xkvp,
                tc.tile_pool(name="ppsum", bufs=4, space="PSUM") as ppsum,
            ):
                wq_t = wpool.tile([P, KO, C], BF16, name="w_sb", tag="wq")
                nc.sync.dma_start(wq_t[:], wq_d.rearrange("k p d -> p k d"))
                wk_t = wpool.tile([P, KO, C], BF16, name="wk_sb", tag="wk")
                nc.sync.dma_start(wk_t[:], wk_d.rearrange("k p d -> p k d"))
                wv_t = wpool.tile([P, KO, C], BF16, name="wv_sb", tag="wv")
                nc.sync.dma_start(wv_t[:], wv_d.rearrange("k p d -> p k d"))

                # ---- phase A: Q projection ----
                for o in range(KO):
                    ps = ppsum.tile([P, TS], F32, name="qk_ps", tag="pp")
                    for ki in range(KO):
                        nc.tensor.matmul(
                            ps[:],
                            wq_t[:, ki, o * P:(o + 1) * P],
                            xq_sb[:, ki, :],
                            start=(ki == 0), stop=(ki == KO - 1),
                        )
                    nc.scalar.activation(
                        q_sb[:, o, :], ps[:],
                        mybir.ActivationFunctionType.Identity,
                        bias=bq_sb[:, o:o + 1])

                # ---- phase B: K and V^T over the full batch, per quarter ----
                with nc.allow_low_precision(reason="bf16/fp8 staging"):
                    for qt in range(4):
                        t0 = qt * TS
                        xh = xkvp.tile([P, KO, TS], BF16,
                                       name="xkv_sb", tag="xkv")
                        nc.sync.dma_start(
                            xh[:],
                            xkv_d.rearrange("(o p) n -> p o n", p=P)
                            [:, :, t0:t0 + TS])
                        for o in range(KO):
                            ps = ppsum.tile([P, TS], F32,
                                            name="qk_ps", tag="pp")
                            for ki in range(KO):
                                nc.tensor.matmul(
                                    ps[:],
                                    wk_t[:, ki, o * P:(o + 1) * P],
                                    xh[:, ki, :],
                                    start=(ki == 0), stop=(ki == KO - 1),
                                )
                            nc.scalar.activation(
                                kq_sb[:, qt, o, :], ps[:],
                                mybir.ActivationFunctionType.Identity,
                                bias=bk_sb[:, o:o + 1])
                        # V^T t-major: psum [t-tile, 512 chans] -> fp8
                        for tt in range(4):
                            for nch in range(2):
                                ps = ppsum.tile([P, TS], F32,
                                                name="qk_ps", tag="pp")
                                for ki in range(KO):
                                    nc.tensor.matmul(
                                        ps[:],
                                        xh[:, ki, tt * P:(tt + 1) * P],
                                        wv_t[:, ki, nch * TS:(nch + 1) * TS],
                                        start=(ki == 0), stop=(ki == KO - 1),
                                    )
                                nc.vector.tensor_copy(
                                    vp_sb[:, qt, tt,
                                          nch * 8:(nch + 1) * 8, :],
                                    ps[:].rearrange("p (h d) -> p h d", d=DH))

            # ---- phase C: attention ----
            with (
                tc.tile_pool(name="wopool", bufs=1) as wopool,
                nc.allow_low_precision(reason="bf16/fp8 attention"),
            ):
                wo_t = wopool.tile([P, KO, C], BF16, name="wo_sb")
                nc.sync.dma_start(wo_t[:], wo_d.rearrange("k p d -> p k d"))
                # residual x fp32 (only consumed by the BN apply at the
                # end; loading it here keeps it off the startup DMA chain)
                nc.sync.dma_start(
                    xqf_sb[:], xqf_d.rearrange("(o p) n -> p o n", p=P))

                attn_pools = (
                    tc.tile_pool(name="epool", bufs=2),
                    tc.tile_pool(name="spsum", bufs=2, space="PSUM"),
                    tc.tile_pool(name="apsum", bufs=1, space="PSUM"),
                    tc.tile_pool(name="bpsum", bufs=1, space="PSUM"),
                )
                epool = attn_pools[0].__enter__()
                spsum = attn_pools[1].__enter__()
                apsum = attn_pools[2].__enter__()
                bpsum = attn_pools[3].__enter__()

                CH = 3  # s-tiles per exp chunk
                chunks = [(s, min(CH, NT - s)) for s in range(0, NT, CH)]

                def emit_sims(h):
                    pb = DH * (h & 1)           # partition base 0/64
                    o = h // 2
                    e_sb = epool.tile([P, NT, TS], FP8, name="e_sb", tag="e")
                    for s0, clen in chunks:
                        ps = spsum.tile([P, CH, TS], F32,
                                        name="sim_ps", tag="sim")
                        for j in range(clen):
                            st = s0 + j
                            nc.tensor.matmul(
                                ps[:, j, :],
                                kq_sb[pb:pb + DH, st // 4, o,
                                      (st % 4) * P:(st % 4 + 1) * P],
                                q_sb[pb:pb + DH, o, :],
                                start=True, stop=True,
                            )
                        nc.scalar.activation(
                            e_sb[:, s0:s0 + clen, :],
                            ps[:, 0:clen, :],
                            mybir.ActivationFunctionType.Exp,
                            scale=SCALE, bias=eshift_sb[:])
                    return e_sb

                def emit_av(h, e_sb, bcden):
                    pb = DH * (h & 1)
                    o = h // 2
                    av = apsum.tile([DH, TS], F32, name="av_ps", tag="av")
                    e2 = e_sb[:].rearrange("p (a j) n -> p a j n", j=2)
                    for a in range(NT // 2):
                        nc.tensor.matmul(
                            av[:],
                            vp_sb[:, a // 2, (a % 2) * 2:(a % 2) * 2 + 2,
                                  h, :],
                            e2[:, a, :, :],
                            start=(a == 0), stop=(a == NT // 2 - 1),
                            perf_mode=DR,
                        )
                    # denominator rides in the bc psum tile at partition 0
                    # (a DoubleRow matmul may not target partition base 64)
                    ds = 0
                    for a in range(NT // 2):
                        nc.tensor.matmul(
                            bcden[ds:ds + 1, :],
                            ones8,
                            e2[:, a, :, :],
                            start=(a == 0), stop=(a == NT // 2 - 1),
                            perf_mode=DR,
                        )
                    nc.vector.tensor_copy(
                        num_sb[pb:pb + DH, o, :], av[:])
                    nc.vector.tensor_copy(
                        den_cat[0:1, h, :], bcden[ds:ds + 1, :])
                    if h & 1:
                        # head pair (2o, 2o+1) done: broadcast raw dens to
                        # 128 partitions by matmul, then invert 128-wide
                        # (a [1,N] single-partition reciprocal runs at 1/128
                        # of DVE rate and stalled the PE ~5us per pair)
                        nc.tensor.matmul(
                            bcden[0:DH, :], onesrow[0:1, 0:DH],
                            den_cat[0:1, h - 1, :], start=True, stop=True)
                        nc.tensor.matmul(
                            bcden[DH:P, :], onesrow[0:1, 0:DH],
                            den_cat[0:1, h, :], start=True, stop=True)
                        nc.vector.reciprocal(rec_sb[:], bcden[:])
                        nc.vector.tensor_tensor(
                            num_sb[:, o, :], num_sb[:, o, :], rec_sb[:],
                            mybir.AluOpType.mult)
                        nc.vector.tensor_scalar_add(
                            num_sb[:, o, :], num_sb[:, o, :],
                            bvc_sb[:, o:o + 1])

                # software-pipelined: AV(h-1) interleaves with sims(h)
                prev = None
                bcden = None
                for h in range(H):
                    e_cur = emit_sims(h)
                    if prev is not None:
                        if prev[0] % 2 == 0:
                            bcden = bpsum.tile([P, TS], F32,
                                               name="bc_ps", tag="bc")
                        emit_av(prev[0], prev[1], bcden)
                    prev = (h, e_cur)
                bcden = bpsum.tile([P, TS], F32, name="bc_ps", tag="bc") \
                    if prev[0] % 2 == 0 else bcden
                emit_av(prev[0], prev[1], bcden)

                for pl in reversed(attn_pools):
                    pl.__exit__(None, None, None)

                # ---- phase E: Wo projection + BN partial stats ----
                with tc.tile_pool(name="ypsum", bufs=4, space="PSUM") as ypsum:
                    for m in range(KO):
                        ps = ypsum.tile([P, TS], F32, name="y_ps", tag="yp")
                        for ki in range(KO):
                            nc.tensor.matmul(
                                ps[:],
                                wo_t[:, ki, m * P:(m + 1) * P],
                                num_sb[:, ki, :],
                                start=(ki == 0), stop=(ki == KO - 1),
                            )
                        nc.scalar.activation(
                            y_sb[:, m, :], ps[:],
                            mybir.ActivationFunctionType.Identity,
                            bias=bo_sb[:, m:m + 1],
                            accum_out=s1_sb[:, m:m + 1])
                        nc.scalar.activation(
                            sq_sb[:], ps[:],
                            mybir.ActivationFunctionType.Square,
                            bias=bo_sb[:, m:m + 1],
                            accum_out=s2_sb[:, m:m + 1])

            # ---- phase F: stats exchange + BN apply + residual ----
            gstats_sb = persist.tile([P, 2 * KO], F32, name="gstats_sb")
            mean_sb = persist.tile([P, KO], F32, name="mean_sb")
            var_sb = persist.tile([P, KO], F32, name="var_sb")
            msq_sb = persist.tile([P, KO], F32, name="msq_sb")
            rstd_sb = persist.tile([P, KO], F32, name="rstd_sb")
            scl_sb = persist.tile([P, KO], F32, name="scl_sb")
            sh_sb = persist.tile([P, KO], F32, name="sh_sb")
            tmp_sb = persist.tile([P, KO, TS], BF16, name="tmp_sb")

            def emit_bn(raw, sA=None, sB=None, sC=None):
                # post-TileContext instructions need concrete (allocated)
                # tensors; inside the context the Tile objects are used
                # directly so the scheduler tracks deps
                cv = ((lambda t: t.tensor.concrete_tensor().ap())
                      if raw else (lambda t: t))
                gst_c = cv(gst_sb)
                gstats_c = cv(gstats_sb)
                mean_c, var_c, msq_c = cv(mean_sb), cv(var_sb), cv(msq_sb)
                rstd_c, scl_c, sh_c = cv(rstd_sb), cv(scl_sb), cv(sh_sb)
                tmp_c, y_c, xqf_c = cv(tmp_sb), cv(y_sb), cv(xqf_sb)
                eps_c, gamma_c, beta_c = cv(eps_sb), cv(gamma_sb), cv(beta_sb)
                nc.vector.reduce_sum(
                    gstats_c[:],
                    gst_c[:].rearrange("p s k -> p k s"),
                    axis=mybir.AxisListType.X)
                nc.vector.tensor_scalar_mul(
                    mean_c[:], gstats_c[:, 0:KO], 1.0 / NBT)
                nc.vector.tensor_scalar_mul(
                    var_c[:], gstats_c[:, KO:2 * KO], 1.0 / NBT)
                nc.vector.tensor_tensor(
                    msq_c[:], mean_c[:], mean_c[:], mybir.AluOpType.mult)
                i = nc.vector.tensor_tensor(
                    var_c[:], var_c[:], msq_c[:], mybir.AluOpType.subtract)
                if raw:
                    i.then_inc(sA, 1)
                    nc.scalar.wait_ge(sA, 1)
                i = nc.scalar.activation(
                    rstd_c[:], var_c[:],
                    mybir.ActivationFunctionType.Sqrt, bias=eps_c[:])
                if raw:
                    i.then_inc(sB, 1)
                    nc.vector.wait_ge(sB, 1)
                nc.vector.reciprocal(rstd_c[:], rstd_c[:])
                nc.vector.tensor_tensor(
                    scl_c[:], gamma_c[:], rstd_c[:], mybir.AluOpType.mult)
                nc.vector.tensor_tensor(
                    sh_c[:], mean_c[:], scl_c[:], mybir.AluOpType.mult)
                nc.vector.tensor_tensor(
                    sh_c[:], beta_c[:], sh_c[:], mybir.AluOpType.subtract)
                outp = out_d.rearrange("(o p) n -> p o n", p=P)
                for m in range(KO):
                    nc.vector.tensor_scalar(
                        tmp_c[:, m, :], y_c[:, m, :],
                        scl_c[:, m:m + 1], sh_c[:, m:m + 1],
                        mybir.AluOpType.mult, mybir.AluOpType.add)
                    i = nc.vector.tensor_tensor(
                        xqf_c[:, m, :], xqf_c[:, m, :], tmp_c[:, m, :],
                        mybir.AluOpType.add)
                    if raw:
                        if m == 3:
                            i.then_inc(sC, 1)
                        elif m == KO - 1:
                            i.then_inc(sC, 1)
                    else:
                        nc.sync.dma_start(outp[:, m, :], xqf_c[:, m, :])
                if raw:
                    nc.sync.wait_ge(sC, 1)
                    nc.sync.dma_start(outp[:, 0:4, :], xqf_c[:, 0:4, :])
                    nc.sync.wait_ge(sC, 2)
                    nc.sync.dma_start(outp[:, 4:KO, :], xqf_c[:, 4:KO, :])

            with nc.allow_low_precision(reason="bn apply"):
                nc.vector.tensor_copy(stats_sb[:, 0:KO], s1_sb[:])
                nc.vector.tensor_copy(stats_sb[:, KO:2 * KO], s2_sb[:])
                if USE_REMOTE_STATS:
                    nc.vector.tensor_copy(gst_sb[:, 0, :], stats_sb[:])
                else:
                    st_in = drampool.tile([P, 2 * KO], F32, name="st_in")
                    st_out = drampool.tile([NCORES, P, 2 * KO], F32,
                                           name="st_out")
                    nc.sync.dma_start(st_in[:], stats_sb[:])
                    nc.gpsimd.collective_compute(
                        "AllGather",
                        mybir.AluOpType.bypass,
                        replica_groups=[list(range(NCORES))],
                        ins=[st_in[:].opt()],
                        outs=[st_out[:].opt()],
                    )
                    nc.sync.dma_start(
                        gst_sb[:], st_out[:].rearrange("s p k -> p s k"))
                    emit_bn(raw=False)

    if USE_REMOTE_STATS:
        # raw tail after the TileContext: the Tile scheduling sim cannot
        # model remotely-incremented semaphores, so the peer-DMA stats
        # exchange and everything depending on it runs in plain program
        # order with manual cross-engine handshakes.
        sA = nc.alloc_semaphore("bn_sA")
        sB = nc.alloc_semaphore("bn_sB")
        sC = nc.alloc_semaphore("bn_sC")
        with nc.allow_low_precision(reason="bn tail"):
            gst_c = gst_sb.tensor.concrete_tensor().ap()
            stats_c = stats_sb.tensor.concrete_tensor().ap()
            nc.gpsimd.bir_kernel_barrier_wait([list(range(NCORES))])
            for j in range(1, NCORES):
                rd = [None] * NCORES
                rd[j] = (0, j)
                nc.gpsimd.remote_dma_broadcast(
                    gst_c[:, j, :], stats_c[:],
                    remote_sem=rsem, local_sem=lsem, rdests=rd)
            nc.gpsimd.trigger_dma(count=NCORES - 1)
            nc.vector.wait_ge(rsem, (NCORES - 1) * 2)
            emit_bn(raw=True, sA=sA, sB=sB, sC=sC)

    nc.compile()
    return nc


def kernel(**inputs) -> np.ndarray:
    global _cached_nc, LAST_RESULT
    import ml_dtypes
    BF = ml_dtypes.bfloat16

    x = np.ascontiguousarray(inputs["x"], dtype=np.float32)
    wT = {k: np.asarray(inputs[k], dtype=np.float32).T
          for k in ("Wq", "Wk", "Wv", "Wo")}
    # host layout [ki, p, d] = W[d, ki*128+p] = W.T reshaped
    w8 = {k: np.ascontiguousarray(v.reshape(KO, P, C).astype(BF))
          for k, v in wT.items()}
    vecs = {k: np.ascontiguousarray(inputs[k], dtype=np.float32)
            for k in ("bq", "bk", "bv", "bo", "gamma", "beta")}

    if _cached_nc is None:
        _cached_nc = _build()
    nc = _cached_nc

    xb = x.astype(BF)
    in_maps = []
    for c in range(NCORES):
        b, t0 = c // 4, TS * (c % 4)
        in_maps.append({
            "xkv": xb[b],
            "xq": np.ascontiguousarray(xb[b][:, t0:t0 + TS]),
            "xqf": np.ascontiguousarray(x[b][:, t0:t0 + TS]),
            "wq": w8["Wq"], "wk": w8["Wk"],
            "wv": w8["Wv"], "wo": w8["Wo"],
            "bq": vecs["bq"], "bk": vecs["bk"], "bv": vecs["bv"],
            "bo": vecs["bo"], "gamma": vecs["gamma"], "beta": vecs["beta"],
        })

    res = run_bass_kernel_spmd(
        nc, in_maps, core_ids=list(range(NCORES)), trace=TRACE)
    LAST_RESULT = res

    out = np.empty((B, C, T), dtype=np.float32)
    for c in range(NCORES):
        b, t0 = c // 4, TS * (c % 4)
        out[b][:, t0:t0 + TS] = res.results[c]["out"]
    return out
